# revision 1
# baseline (speedup 1.0000x reference)
"""Trainium2 Bass kernel for multi-head attention (B=4, N=2048, DIM=1024, H=16, DH=64).

Sharding: 8 cores = 4 batches x 2 query-halves. Each core receives x[b]^T with
its query-half columns rotated to the front (attention is invariant to a
consistent permutation of the key/value axis), computes q for columns 0:1024,
k/v for all 2048, runs scores^T = k_h^T @ q_h per head (row-tiled pairs),
softmax via exp + ones-column denominator folded into the AV matmul, and the
full output projection for its rows. Outputs are disjoint across cores.

Emission interleaves the second half of the projections into the first head
pairs (PE executes in program order, so overlap must be authored), and the
output projection runs two-pass (heads 0-6 early, head pair 7 joined late) to
hide the final softmax-normalize latency.
"""

import os

import numpy as np
import ml_dtypes

import concourse.bass as bass
import concourse.tile as tile
from concourse import bacc, mybir
from concourse import bass_utils

B, N, DIM = 4, 2048, 1024
HEADS, DH = 16, 64
INNER = HEADS * DH
SCALE = DH ** -0.5
NCORES = 8
IH = N // 2          # query rows per core (i-half)
BF16 = mybir.dt.bfloat16
F32 = mybir.dt.float32

KT = DIM // 128          # 8 contraction tiles for projections
NT = N // 128            # 16 j tiles
ES = INNER // 128        # 8 e-slices for q or k
NPP = 8                  # ns groups computed two-pass in phase 3 (all)

_CACHE = {}


def _build_program():
    nc = bacc.Bacc("TRN2", target_bir_lowering=False, debug=False)

    xT_d = nc.dram_tensor("xT", [DIM, N], BF16, kind="ExternalInput")
    wqkv_d = nc.dram_tensor("w_qkv", [DIM, 3 * INNER], BF16, kind="ExternalInput")
    wout_d = nc.dram_tensor("w_out", [INNER, DIM], BF16, kind="ExternalInput")
    bout_d = nc.dram_tensor("b_out", [DIM], F32, kind="ExternalInput")
    out_d = nc.dram_tensor("out", [IH, DIM], F32, kind="ExternalOutput")

    with tile.TileContext(nc) as tc:
        _emit(tc, nc, xT_d, wqkv_d, wout_d, bout_d, out_d)
    nc.compile()
    return nc


def _emit(tc, nc, xT_d, wqkv_d, wout_d, bout_d, out_d):
    from contextlib import ExitStack

    xT_r = xT_d.ap().rearrange("(t p) n -> p t n", p=128)       # [128, 8, 2048]
    w_r = wqkv_d.ap().rearrange("(t p) e -> p t e", p=128)      # [128, 8, 3072]
    wo_r = wout_d.ap().rearrange("(t p) d -> p t d", p=128)     # [128, 8, 1024]

    bap = bout_d.ap()
    bias_bcast = bass.AP(tensor=bap.tensor, offset=bap.offset,
                         ap=[[0, 128]] + [list(d) for d in bap.ap])

    with ExitStack() as ctx:
        consts = ctx.enter_context(tc.tile_pool(name="consts", bufs=1))
        qkv_out = ctx.enter_context(tc.tile_pool(name="qkv_out", bufs=1))
        attn_out = ctx.enter_context(tc.tile_pool(name="attn_out", bufs=1))
        atp = ctx.enter_context(tc.tile_pool(name="attnT", bufs=4))
        rcp = ctx.enter_context(tc.tile_pool(name="rcp", bufs=1))
        bcsp = ctx.enter_context(tc.tile_pool(name="bcs", bufs=2))
        avup = ctx.enter_context(tc.tile_pool(name="avu", bufs=2))
        oddp = ctx.enter_context(tc.tile_pool(name="odd", bufs=1))
        drbp = ctx.enter_context(tc.tile_pool(name="drb", bufs=2, space="DRAM"))
        ps_sc = ctx.enter_context(tc.tile_pool(name="ps_sc", bufs=2, space="PSUM"))

        bias_sb = consts.tile([128, DIM], F32)
        nc.sync.dma_start(out=bias_sb, in_=bias_bcast)
        wo_sb = consts.tile([128, ES, DIM], BF16)       # head pair hp at [:, hp, :]
        nc.sync.dma_start(out=wo_sb, in_=wo_r)

        qTs = [qkv_out.tile([128, IH], BF16, name=f"qT{s}") for s in range(ES)]
        kTs = [qkv_out.tile([128, N], BF16, name=f"kT{s}") for s in range(ES)]
        v_lo = qkv_out.tile([128, NT, 8, DH + 1], BF16)  # heads 0-7 (+ones col)
        v_hi = qkv_out.tile([128, NT, 8, DH + 1], BF16)  # heads 8-15
        nc.vector.memset(v_lo[:, :, :, DH], 1.0)
        nc.vector.memset(v_hi[:, :, :, DH], 1.0)
        aoTs = [attn_out.tile([128, IH], BF16, name=f"aoT{s}") for s in range(ES)]

        # ---- projection group emitters (psum from a given pool/tag) ----
        def q_slice(pool, tag, wg, s4, s):
            ps = pool.tile([128, IH], F32, tag=tag, name=f"q_ps{s}")
            for c in range(IH // 512):
                for k in range(KT):
                    nc.tensor.matmul(
                        ps[:, 512 * c:512 * (c + 1)],
                        wg[:, k, 128 * s4:128 * (s4 + 1)],
                        xTk[k][:, 512 * c:512 * (c + 1)],
                        start=(k == 0), stop=(k == KT - 1))
            nc.vector.tensor_copy(out=qTs[s], in_=ps)

        def k_slice(pool, tag, wg, s4, s, half):
            ps = pool.tile([128, IH], F32, tag=tag, name=f"k_ps{s}_{half}")
            for c in range(IH // 512):
                for k in range(KT):
                    nc.tensor.matmul(
                        ps[:, 512 * c:512 * (c + 1)],
                        wg[:, k, 128 * s4:128 * (s4 + 1)],
                        xTk[k][:, IH * half + 512 * c:IH * half + 512 * (c + 1)],
                        start=(k == 0), stop=(k == KT - 1))
            nc.vector.tensor_copy(
                out=kTs[s][:, IH * half:IH * (half + 1)], in_=ps)

        def v_tile(pool, tag, wg, vdst, t):
            ps = pool.tile([128, 512], F32, tag=tag, name=f"v_ps{t}")
            for k in range(KT):
                nc.tensor.matmul(
                    ps, xTk[k][:, 128 * t:128 * (t + 1)], wg[:, k, :],
                    start=(k == 0), stop=(k == KT - 1))
            nc.vector.tensor_copy(
                out=vdst[:, t, :, 0:DH],
                in_=ps.rearrange("p (h d) -> p h d", h=8))

        # ---- head-pair emitter with optional per-step filler ----
        def pair(s, ps_av, filler=None):
            av0 = ps_av.tile([DH + 1, IH], F32, tag="av", name=f"av0_{s}")
            av1 = ps_av.tile([DH + 1, IH], F32, tag="av", name=f"av1_{s}")
            avs = [av0, av1]
            step = 0
            for t in range(NT):
                for p in range(2):
                    h = 2 * s + p
                    pb = 64 * p
                    sc = ps_sc.tile([128, IH], F32, tag="sc", name=f"sc{s}_{t}_{p}")
                    for c in range(IH // 512):
                        nc.tensor.matmul(
                            sc[:, 512 * c:512 * (c + 1)],
                            kTs[s][pb:pb + 64, 128 * t:128 * (t + 1)],
                            qTs[s][pb:pb + 64, 512 * c:512 * (c + 1)],
                            start=True, stop=True, tile_position=(pb, 0))
                    at = atp.tile([128, IH], BF16, tag="at", name=f"at{s}_{t}_{p}")
                    nc.scalar.activation(
                        out=at, in_=sc,
                        func=mybir.ActivationFunctionType.Exp, scale=SCALE)
                    vsrc = v_lo if h < 8 else v_hi
                    for c in range(IH // 512):
                        nc.tensor.matmul(
                            avs[p][:, 512 * c:512 * (c + 1)],
                            vsrc[:, t, h % 8, :],
                            at[:, 512 * c:512 * (c + 1)],
                            start=(t == 0), stop=(t == NT - 1))
                    if filler is not None:
                        filler(step)
                    step += 1
            for p in range(2):
                av = avs[p]
                avu = avup.tile([DH + 1, IH], F32, tag="avu", name=f"avu{s}_{p}")
                nc.vector.tensor_copy(out=avu, in_=av)
                rc = rcp.tile([128, IH], BF16, tag="rc", name=f"rc{s}_{p}")
                with nc.allow_low_precision(reason="softmax denom recip in bf16"):
                    nc.vector.reciprocal(
                        out=rc[DH:DH + 1, :], in_=avu[DH:DH + 1, :])
                dr = drbp.tile([IH], BF16, tag="dr", name=f"dr{s}_{p}")
                nc.sync.dma_start(out=dr, in_=rc[DH:DH + 1, :])
                dr_bc = bass.AP(tensor=dr.tensor, offset=dr.offset,
                                ap=[[0, DH]] + [list(dd) for dd in dr.ap])
                bcs = bcsp.tile([DH, IH], BF16, tag="bcs", name=f"bcs{s}_{p}")
                nc.sync.dma_start(out=bcs, in_=dr_bc)
                if p == 0:
                    nc.vector.tensor_mul(
                        out=aoTs[s][0:DH, :], in0=avu[0:DH, :], in1=bcs)
                else:
                    od = oddp.tile([DH, IH], BF16, tag="od", name=f"od{s}")
                    nc.vector.tensor_mul(out=od, in0=avu[0:DH, :], in1=bcs)
                    nc.sync.dma_start(out=aoTs[s][DH:128, :], in_=od)

        # ---------------- phase 1a: v_lo, k0-3, q0-3 ----------------
        p1ctx = ExitStack()
        p1x = p1ctx.enter_context(tc.tile_pool(name="p1_x", bufs=1))
        p1w = p1ctx.enter_context(tc.tile_pool(name="p1_w", bufs=2))
        xTk = [p1x.tile([128, N], BF16, name=f"xTk{k}") for k in range(KT)]
        for k in range(KT):
            eng = nc.sync if k % 2 == 0 else nc.gpsimd
            eng.dma_start(out=xTk[k], in_=xT_r[:, k, :])

        wgs = {}
        for g in (4, 2, 0, 5, 3, 1):
            wgs[g] = p1w.tile([128, KT, 512], BF16, tag="wg", name=f"wg{g}")
        with tc.tile_pool(name="p1_ps", bufs=2, space="PSUM") as p1ps:
            nc.sync.dma_start(out=wgs[4], in_=w_r[:, :, 2048:2560])
            for t in range(NT):
                v_tile(p1ps, "ps", wgs[4], v_lo, t)
            nc.sync.dma_start(out=wgs[2], in_=w_r[:, :, 1024:1536])
            for s4 in range(4):
                for half in range(2):
                    k_slice(p1ps, "ps", wgs[2], s4, s4, half)
            nc.sync.dma_start(out=wgs[0], in_=w_r[:, :, 0:512])
            for s4 in range(4):
                q_slice(p1ps, "ps", wgs[0], s4, s4)

        # ------------ phase 2 pairs 0-3, with 1b sprinkled ------------
        ps_av = ctx.enter_context(tc.tile_pool(name="ps_av", bufs=2, space="PSUM"))
        if True:
                pair(0, ps_av)

                pair(1, ps_av)

                # second half of projections between pairs 1 and 2
                # (borrows the idle scores-psum slots; no QK runs during this block)
                nc.sync.dma_start(out=wgs[5], in_=w_r[:, :, 2560:3072])
                for t in range(NT):
                    v_tile(ps_sc, "sc", wgs[5], v_hi, t)
                nc.sync.dma_start(out=wgs[3], in_=w_r[:, :, 1536:2048])
                for s4 in range(4):
                    for half in range(2):
                        k_slice(ps_sc, "sc", wgs[3], s4, 4 + s4, half)
                nc.sync.dma_start(out=wgs[1], in_=w_r[:, :, 512:1024])
                for s4 in range(4):
                    q_slice(ps_sc, "sc", wgs[1], s4, 4 + s4)

                for s in range(2, 4):
                    pair(s, ps_av)

                p1ctx.close()   # free xT + w staging before pairs 4-7

                for s in range(4, ES):
                    pair(s, ps_av)

                # ---------------- phase 3: output projection ----------------
                with tc.tile_pool(name="p3_st", bufs=2) as p3st, \
                     tc.tile_pool(name="p3_pp", bufs=NPP) as p3pp:
                    pps = []
                    for ns in range(NPP):   # pass 1: heads 0-13 + bias
                        po = ps_av.tile([128, DIM], F32, tag="av", name=f"po{ns}")
                        for c in range(DIM // 512):
                            for hp in range(ES - 1):
                                nc.tensor.matmul(
                                    po[:, 512 * c:512 * (c + 1)],
                                    aoTs[hp][:, 128 * ns:128 * (ns + 1)],
                                    wo_sb[:, hp, 512 * c:512 * (c + 1)],
                                    start=(hp == 0), stop=(hp == ES - 2))
                        pp = p3pp.tile([128, DIM], F32, tag="pp", name=f"pp{ns}")
                        nc.vector.tensor_add(out=pp, in0=po, in1=bias_sb)
                        pps.append(pp)
                    for ns in range(NPP):   # pass 2: join head pair 7
                        po = ps_av.tile([128, DIM], F32, tag="av", name=f"po2_{ns}")
                        for c in range(DIM // 512):
                            nc.tensor.matmul(
                                po[:, 512 * c:512 * (c + 1)],
                                aoTs[ES - 1][:, 128 * ns:128 * (ns + 1)],
                                wo_sb[:, ES - 1, 512 * c:512 * (c + 1)],
                                start=True, stop=True)
                        st = p3st.tile([128, DIM], F32, tag="st", name=f"st{ns}")
                        nc.vector.tensor_add(out=st, in0=po, in1=pps[ns])
                        nc.sync.dma_start(
                            out=out_d.ap()[128 * ns:128 * (ns + 1), :], in_=st)
                    for ns in range(NPP, IH // 128):   # remaining: single pass
                        po = ps_av.tile([128, DIM], F32, tag="av", name=f"po1_{ns}")
                        for c in range(DIM // 512):
                            for hp in range(ES):
                                nc.tensor.matmul(
                                    po[:, 512 * c:512 * (c + 1)],
                                    aoTs[hp][:, 128 * ns:128 * (ns + 1)],
                                    wo_sb[:, hp, 512 * c:512 * (c + 1)],
                                    start=(hp == 0), stop=(hp == ES - 1))
                        st = p3st.tile([128, DIM], F32, tag="st", name=f"st{ns}")
                        nc.vector.tensor_add(out=st, in0=po, in1=bias_sb)
                        nc.sync.dma_start(
                            out=out_d.ap()[128 * ns:128 * (ns + 1), :], in_=st)


def get_program():
    if "nc" not in _CACHE:
        _CACHE["nc"] = _build_program()
    return _CACHE["nc"]


def make_in_maps(x, w_qkv, w_out, b_out):
    bf = ml_dtypes.bfloat16
    w_qkv_b = np.ascontiguousarray(w_qkv, np.float32).astype(bf)
    w_out_b = np.ascontiguousarray(w_out, np.float32).astype(bf)
    b_out_f = np.ascontiguousarray(b_out, np.float32)
    in_maps = []
    for core in range(NCORES):
        b, half = core // 2, core % 2
        xT = np.ascontiguousarray(np.asarray(x[b], np.float32).T).astype(bf)
        if half == 1:   # rotate this core's query half to the front
            xT = np.concatenate([xT[:, IH:], xT[:, :IH]], axis=1)
        in_maps.append({
            "xT": np.ascontiguousarray(xT),
            "w_qkv": w_qkv_b,
            "w_out": w_out_b,
            "b_out": b_out_f,
        })
    return in_maps


def kernel(x, w_qkv, w_out, b_out):
    nc = get_program()
    in_maps = make_in_maps(x, w_qkv, w_out, b_out)
    res = bass_utils.run_bass_kernel_spmd(nc, in_maps, core_ids=list(range(NCORES)))
    out = np.empty((B, N, DIM), np.float32)
    for core in range(NCORES):
        b, half = core // 2, core % 2
        out[b, IH * half:IH * (half + 1), :] = res.results[core]["out"]
    return out



# revision 6
# speedup vs baseline: 1.2749x; 1.2749x over previous
"""Trainium2 Bass kernel for multi-head attention (B=4, N=2048, DIM=1024, H=16, DH=64).

Sharding: 8 cores = 4 batches x 2 head-groups (8 heads each, column-parallel
qkv / row-parallel out-proj).  Each core computes q/k/v for its 8 heads over
the full 2048 queries, attention per head-pair with PE tile packing, and a
PARTIAL output projection (its 512 rows of w_out); the host gather sums the
two partial outputs per batch.

PE schedule: mode-homogeneous streaks to avoid tile-mode drains.
  - scores: (64,128) row-tiled T0/T8 head pairs, 2x concurrent
  - AV + denominator: (128,64) col-tiled T0/T1, 2x concurrent; denominators
    via ones[128,64] matmul -> 64-row broadcast for free
  - softmax exp on ScalarE over [128,1024] two-head psum macro tiles
  - normalize: reciprocal_approx_fast (DVE custom op) + one tensor_mul
QKV projection chunks are interleaved into pair 0's attention so the exp
engine (the end-to-end bottleneck) starts as early as possible.
"""

import numpy as np
import ml_dtypes

import concourse.bass as bass
import concourse.tile as tile
from concourse import bacc, mybir
from concourse import bass_utils

B, N, DIM = 4, 2048, 1024
HEADS, DH = 16, 64
INNER = HEADS * DH
SCALE = DH ** -0.5
NCORES = 8
HG = 8                    # heads per core
NP = HG // 2              # head pairs per core
KT = DIM // 128           # 8 contraction tiles for projections
NT = N // 128             # 16 j tiles
NC = N // 512             # 4 i chunks
BF16 = mybir.dt.bfloat16
F32 = mybir.dt.float32

_CACHE = {}


def _build_program():
    nc = bacc.Bacc("TRN2", target_bir_lowering=False, debug=False)
    xT_d = nc.dram_tensor("xT", [DIM, N], BF16, kind="ExternalInput")
    wq_d = nc.dram_tensor("wq", [DIM, 512], BF16, kind="ExternalInput")
    wk_d = nc.dram_tensor("wk", [DIM, 512], BF16, kind="ExternalInput")
    wv_d = nc.dram_tensor("wv", [DIM, 512], BF16, kind="ExternalInput")
    wo_d = nc.dram_tensor("wo", [512, DIM], BF16, kind="ExternalInput")
    bias_d = nc.dram_tensor("bias", [DIM], F32, kind="ExternalInput")
    out_d = nc.dram_tensor("out", [N, DIM], F32, kind="ExternalOutput")
    with tile.TileContext(nc) as tc:
        _emit(tc, nc, xT_d, wq_d, wk_d, wv_d, wo_d, bias_d, out_d)
    nc.compile()
    return nc


def _emit(tc, nc, xT_d, wq_d, wk_d, wv_d, wo_d, bias_d, out_d):
    from contextlib import ExitStack

    xT_r = xT_d.ap().rearrange("(t p) n -> p t n", p=128)    # [128, 8, 2048]
    wq_r = wq_d.ap().rearrange("(t p) e -> p t e", p=128)    # [128, 8, 512]
    wk_r = wk_d.ap().rearrange("(t p) e -> p t e", p=128)
    wv_r = wv_d.ap().rearrange("(t p) e -> p t e", p=128)
    wo_r = wo_d.ap().rearrange("(t p) d -> p t d", p=128)    # [128, 4, 1024]
    bap = bias_d.ap()
    bias_bc = bass.AP(tensor=bap.tensor, offset=bap.offset,
                      ap=[[0, 128]] + [list(d) for d in bap.ap])

    with ExitStack() as ctx:
        consts = ctx.enter_context(tc.tile_pool(name="consts", bufs=1))
        qkv = ctx.enter_context(tc.tile_pool(name="qkv", bufs=1))
        atp = ctx.enter_context(tc.tile_pool(name="atp", bufs=2))
        recp = ctx.enter_context(tc.tile_pool(name="recp", bufs=2))
        outp = ctx.enter_context(tc.tile_pool(name="outp", bufs=2))
        psc = ctx.enter_context(tc.tile_pool(name="psc", bufs=2, space="PSUM"))
        pmix = ctx.enter_context(tc.tile_pool(name="pmix", bufs=2, space="PSUM"))

        stg = ExitStack()
        xp = stg.enter_context(tc.tile_pool(name="xp", bufs=1))
        wp = stg.enter_context(tc.tile_pool(name="wp", bufs=1))
        wqk = stg.enter_context(tc.tile_pool(name="wqk", bufs=4))

        # ---- staged inputs ----
        xTk = xp.tile([128, KT, N], BF16)
        for k in range(KT):
            eng = nc.sync if k % 2 == 0 else nc.gpsimd
            eng.dma_start(out=xTk[:, k, :], in_=xT_r[:, k, :])
        wv_sb = wp.tile([128, KT, 512], BF16)
        nc.sync.dma_start(out=wv_sb, in_=wv_r)

        def w_slice(w_r, s, nm):
            t = wqk.tile([128, KT, 128], BF16, tag="w", name=nm)
            nc.sync.dma_start(out=t, in_=w_r[:, :, 128 * s:128 * (s + 1)])
            return t

        # ---- persistent ----
        ones = consts.tile([128, 64], BF16)
        nc.vector.memset(ones, 1.0)
        wo_sb = consts.tile([128, 4, DIM], BF16)
        nc.sync.dma_start(out=wo_sb, in_=wo_r)
        bias_sb = consts.tile([128, DIM], F32)
        nc.sync.dma_start(out=bias_sb, in_=bias_bc)
        qT = qkv.tile([128, NP, N], BF16)     # pair s rows: head 2s | 2s+1
        kT = qkv.tile([128, NP, N], BF16)
        v = qkv.tile([128, NT, HG, DH], BF16)
        aoT = qkv.tile([128, NP, N], BF16)    # normalized attn out (e rows)

        # ---- emitters ----
        def proj(dst, w_sb, s):
            # one pair-slice of q or k: out rows = 128 (2 heads), all i
            for ih in range(2):
                ps = pmix.tile([128, 1024], F32, tag="mix", name=f"pj{s}_{ih}")
                for half in range(2):
                    sl = slice(512 * half, 512 * (half + 1))
                    for k in range(KT):
                        nc.tensor.matmul(
                            ps[:, sl],
                            w_sb[:, k, :],
                            xTk[:, k, 1024 * ih + 512 * half:
                                1024 * ih + 512 * (half + 1)],
                            start=(k == 0), stop=(k == KT - 1))
                nc.vector.tensor_copy(
                    out=dst[:, s, 1024 * ih:1024 * (ih + 1)], in_=ps)

        def v_tiles():
            for t in range(0, NT, 2):
                ps = pmix.tile([128, 1024], F32, tag="mix", name=f"v{t}")
                for tt in range(2):
                    for k in range(KT):
                        nc.tensor.matmul(
                            ps[:, 512 * tt:512 * (tt + 1)],
                            xTk[:, k, 128 * (t + tt):128 * (t + tt + 1)],
                            wv_sb[:, k, :], start=(k == 0), stop=(k == KT - 1))
                nc.vector.tensor_copy(
                    out=v[:, t:t + 2, :, :],
                    in_=ps.rearrange("p (t2 h d) -> p t2 h d", t2=2, h=HG))

        def sc_streak(s, c, at):
            # scores^T for pair s, i-chunk c: (64,128) T0/T8 packed
            isl = slice(512 * c, 512 * (c + 1))
            for t in range(NT):
                sc = psc.tile([128, 1024], F32, tag="sc", name=f"sc{s}_{c}_{t}")
                nc.tensor.matmul(
                    sc[:, 0:512], kT[0:64, s, 128 * t:128 * (t + 1)],
                    qT[0:64, s, isl], start=True, stop=True)
                nc.tensor.matmul(
                    sc[:, 512:1024], kT[64:128, s, 128 * t:128 * (t + 1)],
                    qT[64:128, s, isl], start=True, stop=True)
                nc.scalar.activation(
                    out=at[:, t, :], in_=sc,
                    func=mybir.ActivationFunctionType.Exp, scale=SCALE)

        def av_streak(s, c, at):
            # AV + denominator, (128,64) col-tiled T0/T1; den = ones matmul
            avd = pmix.tile([128, 1024], F32, tag="mix", name=f"avd{s}_{c}")
            for t in range(NT):
                st, sp = (t == 0), (t == NT - 1)
                nc.tensor.matmul(avd[0:64, 0:512], v[:, t, 2 * s, :],
                                 at[:, t, 0:512], start=st, stop=sp)
                nc.tensor.matmul(avd[64:128, 0:512], v[:, t, 2 * s + 1, :],
                                 at[:, t, 512:1024], start=st, stop=sp)
                nc.tensor.matmul(avd[0:64, 512:1024], ones,
                                 at[:, t, 0:512], start=st, stop=sp)
                nc.tensor.matmul(avd[64:128, 512:1024], ones,
                                 at[:, t, 512:1024], start=st, stop=sp)
            rec = recp.tile([128, 512], F32, tag="rec", name=f"rec{s}_{c}")
            nc.vector.reciprocal_approx_fast(out=rec, in_=avd[:, 512:1024])
            nc.vector.tensor_mul(
                out=aoT[:, s, 512 * c:512 * (c + 1)],
                in0=avd[:, 0:512], in1=rec)

        def outproj(c):
            # output rows 512c..512c+512 (4 i-tiles), full 1024 cols
            for it in range(4 * c, 4 * c + 4):
                po = pmix.tile([128, 1024], F32, tag="mix", name=f"po{it}")
                for half in range(2):
                    sl = slice(512 * half, 512 * (half + 1))
                    for s in range(NP):
                        nc.tensor.matmul(
                            po[:, sl], aoT[:, s, 128 * it:128 * (it + 1)],
                            wo_sb[:, s, sl], start=(s == 0), stop=(s == NP - 1))
                ot = outp.tile([128, DIM], F32, tag="out", name=f"ot{it}")
                nc.vector.tensor_add(out=ot, in0=po, in1=bias_sb)
                nc.sync.dma_start(
                    out=out_d.ap()[128 * it:128 * (it + 1), :], in_=ot)

        # ---- phase scheduling: c-outer so out-proj overlaps later exps ----
        ats = {}

        def at_tile(s, c):
            t = atp.tile([128, NT, 1024], BF16, tag="at", name=f"at{s}_{c}")
            ats[(s, c)] = t
            return t

        wqs = w_slice(wq_r, 0, "wq0")
        wks = w_slice(wk_r, 0, "wk0")
        proj(qT, wqs, 0)
        proj(kT, wks, 0)
        sc_streak(0, 0, at_tile(0, 0))
        v_tiles()
        av_streak(0, 0, ats[(0, 0)])
        for s in range(1, NP):
            wqs = w_slice(wq_r, s, f"wq{s}")
            wks = w_slice(wk_r, s, f"wk{s}")
            proj(qT, wqs, s)
            proj(kT, wks, s)
            sc_streak(s, 0, at_tile(s, 0))
            av_streak(s, 0, ats[(s, 0)])
        stg.close()
        outproj(0)
        for c in range(1, NC):
            for s in range(NP):
                sc_streak(s, c, at_tile(s, c))
                av_streak(s, c, ats[(s, c)])
            outproj(c)


def get_program():
    if "nc" not in _CACHE:
        _CACHE["nc"] = _build_program()
    return _CACHE["nc"]


def make_in_maps(x, w_qkv, w_out, b_out):
    bf = ml_dtypes.bfloat16
    w = np.ascontiguousarray(w_qkv, np.float32)
    wo = np.ascontiguousarray(w_out, np.float32)
    b = np.ascontiguousarray(b_out, np.float32)
    zeros = np.zeros_like(b)
    xTs = [np.ascontiguousarray(np.asarray(x[bb], np.float32).T).astype(bf)
           for bb in range(B)]
    in_maps = []
    for core in range(NCORES):
        bb, hg = core // 2, core % 2
        cs = slice(512 * hg, 512 * (hg + 1))
        in_maps.append({
            "xT": xTs[bb],
            "wq": np.ascontiguousarray(w[:, cs]).astype(bf),
            "wk": np.ascontiguousarray(w[:, 1024:2048][:, cs]).astype(bf),
            "wv": np.ascontiguousarray(w[:, 2048:3072][:, cs]).astype(bf),
            "wo": np.ascontiguousarray(wo[cs, :]).astype(bf),
            "bias": b if hg == 0 else zeros,
        })
    return in_maps


def kernel(x, w_qkv, w_out, b_out):
    nc = get_program()
    in_maps = make_in_maps(x, w_qkv, w_out, b_out)
    res = bass_utils.run_bass_kernel_spmd(nc, in_maps, core_ids=list(range(NCORES)))
    out = np.empty((B, N, DIM), np.float32)
    for bb in range(B):
        out[bb] = res.results[2 * bb]["out"] + res.results[2 * bb + 1]["out"]
    return out


# revision 11
# speedup vs baseline: 1.3469x; 1.0565x over previous
"""Trainium2 Bass kernel for multi-head attention (B=4, N=2048, DIM=1024, H=16, DH=64).

Sharding: 8 cores = 4 batches x 2 head-groups (8 heads each): column-parallel
qkv, row-parallel out-proj.  Each core computes q/k/v for its 8 heads over the
full 2048 queries, attention per head-pair with PE tile packing, and a PARTIAL
output projection; the host gather sums the two partial outputs per batch and
adds the bias.

PE schedule: software-pipelined units. Unit (s,c) interleaves, in groups of 3
j-tiles, the scores matmuls of (s,c) [(64,128) row-tiled T0/T8 pairs, 2x
concurrent] with the AV + denominator matmuls of the previous unit [(128,64)
col-tiled T0/T1, 2x concurrent; denominators via ones[128,64] matmul give a
64-row broadcast for free].  Softmax exp runs on ScalarE for 12/16 j-tiles and
on the DVE for 4/16 via a 2-pass bit-trick exp (floor extract + quadratic
mantissa correction, ~0.3% shape error, uniform scale cancels in softmax).
Normalize = reciprocal_approx_fast + one tensor_mul.  QKV projection chunks
ride as PE fillers inside units; out-proj runs as the epilogue.
"""

import numpy as np
import ml_dtypes

import concourse.bass as bass
import concourse.tile as tile
from concourse import bacc, mybir
from concourse import bass_utils

B, N, DIM = 4, 2048, 1024
HEADS, DH = 16, 64
INNER = HEADS * DH
SCALE = DH ** -0.5
NCORES = 8
HG = 8                    # heads per core
NP = HG // 2              # head pairs per core
KT = DIM // 128           # contraction tiles for projections
NT = N // 128             # 16 j tiles
NC = N // 512             # 4 i chunks
BF16 = mybir.dt.bfloat16
F32 = mybir.dt.float32
I16 = mybir.dt.int16

EXPA = float(np.float32(SCALE / np.log(2)))   # y = score * EXPA = log2(e^(s*SCALE))
EXPD = -0.34                                  # quadratic mantissa correction coeff
DVESET = (3, 7, 11, 15)                       # j-tiles exp'd on DVE (bit-trick)

_CACHE = {}


def _make_exp16():
    """Register the 2nd-pass exp op (quadratic-corrected Schraudolph ->
    bf16 bits) in the custom-DVE registry, reusing a spare opcode row."""
    from concourse.dve_spec import (
        Spec, Src0, Src1, C0, C1, C2, One, lower, _has_src1)
    from concourse.dve_uop import DveOpSpec
    from concourse import dve_ops
    from concourse.dve_ops import DveOp, get_dve_sub_opcode

    name = "CODY_WAITE_CASCADE"
    y = Src0 * C0
    f = (y - Src1) + (C2 - One)
    m = f * ((One + C1) - C1 * f)
    body = (Src1 + m) * C2

    def ref(in0, in1, s0, s1, imm2):
        yy = in0.astype(np.float32) * s0
        ff = (yy - in1) + (imm2 - 1.0)
        mm = ff * ((1.0 + s1) - s1 * ff)
        return (in1 + mm) * imm2

    spec = Spec(body=body, reference=ref)
    shas = {}
    for ver in ("v3", "v4"):
        uops = lower(spec, ver=ver)
        shas[ver] = DveOpSpec(name=name, opcode=get_dve_sub_opcode(name),
                              uops=uops, rd1_en=_has_src1(spec)).sha(ver)
    op = DveOp(name, spec, subdim=False, uops_sha=shas)
    dve_ops.OPS.append(op)
    return op


EXP16 = _make_exp16()


def _build_program():
    nc = bacc.Bacc("TRN2", target_bir_lowering=False, debug=False)
    xT_d = nc.dram_tensor("xT", [DIM, N], BF16, kind="ExternalInput")
    wq_d = nc.dram_tensor("wq", [DIM, 512], BF16, kind="ExternalInput")
    wk_d = nc.dram_tensor("wk", [DIM, 512], BF16, kind="ExternalInput")
    wv_d = nc.dram_tensor("wv", [DIM, 512], BF16, kind="ExternalInput")
    wo_d = nc.dram_tensor("wo", [512, DIM], BF16, kind="ExternalInput")
    out_d = nc.dram_tensor("out", [N, DIM], BF16, kind="ExternalOutput")
    with tile.TileContext(nc) as tc:
        _emit(tc, nc, xT_d, wq_d, wk_d, wv_d, wo_d, out_d)
    nc.compile()
    return nc


def _emit(tc, nc, xT_d, wq_d, wk_d, wv_d, wo_d, out_d):
    from contextlib import ExitStack

    xT_r = xT_d.ap().rearrange("(t p) n -> p t n", p=128)    # [128, 8, 2048]
    wq_r = wq_d.ap().rearrange("(t p) e -> p t e", p=128)    # [128, 8, 512]
    wk_r = wk_d.ap().rearrange("(t p) e -> p t e", p=128)
    wv_r = wv_d.ap().rearrange("(t p) e -> p t e", p=128)
    wo_r = wo_d.ap().rearrange("(t p) d -> p t d", p=128)    # [128, 4, 1024]

    with ExitStack() as ctx:
        consts = ctx.enter_context(tc.tile_pool(name="consts", bufs=1))
        qkv = ctx.enter_context(tc.tile_pool(name="qkv", bufs=1))
        atp = ctx.enter_context(tc.tile_pool(name="atp", bufs=16))
        up = ctx.enter_context(tc.tile_pool(name="up", bufs=4))
        recp = ctx.enter_context(tc.tile_pool(name="recp", bufs=2))
        outp = ctx.enter_context(tc.tile_pool(name="outp", bufs=2))
        xp = ctx.enter_context(tc.tile_pool(name="xp", bufs=1))
        wp = ctx.enter_context(tc.tile_pool(name="wp", bufs=1))
        wqk = ctx.enter_context(tc.tile_pool(name="wqk", bufs=4))
        psc = ctx.enter_context(tc.tile_pool(name="psc", bufs=3, space="PSUM"))
        pwk = ctx.enter_context(tc.tile_pool(name="pwk", bufs=2, space="PSUM"))

        # ---- input DMAs, spread over 4 queues; first-needed first ----
        wqs = {0: wqk.tile([128, KT, 128], BF16, tag="w", name="wq0")}
        wks = {0: wqk.tile([128, KT, 128], BF16, tag="w", name="wk0")}
        nc.sync.dma_start(out=wqs[0], in_=wq_r[:, :, 0:128])
        nc.sync.dma_start(out=wks[0], in_=wk_r[:, :, 0:128])
        xTk = xp.tile([128, KT, N], BF16)
        qs = [nc.sync, nc.gpsimd, nc.scalar]
        for k in range(KT):
            qs[k % 3].dma_start(out=xTk[:, k, :], in_=xT_r[:, k, :])
        wv_sb = wp.tile([128, KT, 512], BF16)
        nc.gpsimd.dma_start(out=wv_sb, in_=wv_r)
        wo_sb = consts.tile([128, 4, DIM], BF16)
        nc.scalar.dma_start(out=wo_sb, in_=wo_r)
        ones = consts.tile([128, 64], BF16)
        nc.vector.memset(ones, 1.0)

        # ---- persistent sbuf ----
        qT = qkv.tile([128, NP, N], BF16)     # pair s rows: head 2s | 2s+1
        kT = qkv.tile([128, NP, N], BF16)
        v = qkv.tile([128, NT, HG, DH], BF16)
        aoT = qkv.tile([128, NP, N], BF16)

        # ---- emitters ----
        def w_fetch(s):
            wqs[s] = wqk.tile([128, KT, 128], BF16, tag="w", name=f"wq{s}")
            wks[s] = wqk.tile([128, KT, 128], BF16, tag="w", name=f"wk{s}")
            nc.sync.dma_start(out=wqs[s], in_=wq_r[:, :, 128 * s:128 * (s + 1)])
            nc.sync.dma_start(out=wks[s], in_=wk_r[:, :, 128 * s:128 * (s + 1)])

        def proj_tile(dst, w_sb, s, ih):
            # [128 e, 1024 i] chunk of q or k pair-slice s
            ps = psc.tile([128, 1024], F32, tag="sc", name=f"pj{dst is kT}_{s}_{ih}")
            for half in range(2):
                sl = slice(512 * half, 512 * (half + 1))
                for k in range(KT):
                    nc.tensor.matmul(
                        ps[:, sl], w_sb[:, k, :],
                        xTk[:, k, 1024 * ih + 512 * half:
                            1024 * ih + 512 * (half + 1)],
                        start=(k == 0), stop=(k == KT - 1))
            nc.vector.tensor_copy(
                out=dst[:, s, 1024 * ih:1024 * (ih + 1)], in_=ps)

        def v_tile(j):
            # v j-tile pair (2j, 2j+1) for all 8 heads
            ps = psc.tile([128, 1024], F32, tag="sc", name=f"v{j}")
            for tt in range(2):
                for k in range(KT):
                    nc.tensor.matmul(
                        ps[:, 512 * tt:512 * (tt + 1)],
                        xTk[:, k, 128 * (2 * j + tt):128 * (2 * j + tt + 1)],
                        wv_sb[:, k, :], start=(k == 0), stop=(k == KT - 1))
            nc.vector.tensor_copy(
                out=v[:, 2 * j:2 * j + 2, :, :],
                in_=ps.rearrange("p (t2 h d) -> p t2 h d", t2=2, h=HG))

        ats = {}

        def sc_tile(s, c, t):
            # scores^T pair s, i-chunk c, j-tile t + exp dispatch
            isl = slice(512 * c, 512 * (c + 1))
            sc = psc.tile([128, 1024], F32, tag="sc", name=f"sc{s}_{c}_{t}")
            nc.tensor.matmul(
                sc[:, 0:512], kT[0:64, s, 128 * t:128 * (t + 1)],
                qT[0:64, s, isl], start=True, stop=True)
            nc.tensor.matmul(
                sc[:, 512:1024], kT[64:128, s, 128 * t:128 * (t + 1)],
                qT[64:128, s, isl], start=True, stop=True)
            at = atp.tile([128, 1024], BF16, tag="at", name=f"at{s}_{c}_{t}")
            ats[(s, c, t)] = at
            if t in DVESET:
                u = up.tile([128, 1024], I16, tag="u", name=f"u{s}_{c}_{t}")
                for hf in range(2):
                    sl = slice(512 * hf, 512 * (hf + 1))
                    nc.vector.tensor_scalar(
                        out=u[:, sl], in0=sc[:, sl], scalar1=EXPA, scalar2=126.5,
                        op0=mybir.AluOpType.mult, op1=mybir.AluOpType.add)
                    nc.vector._custom_dve(
                        EXP16, out=at.bitcast(I16)[:, sl], in0=sc[:, sl],
                        in1=u[:, sl], s0=EXPA, s1=EXPD, imm2=128.0)
            else:
                nc.scalar.activation(
                    out=at, in_=sc,
                    func=mybir.ActivationFunctionType.Exp, scale=SCALE)

        def avden_alloc(s, c):
            av = pwk.tile([128, 512], F32, tag="wk", name=f"av{s}_{c}")
            den = pwk.tile([128, 512], F32, tag="wk", name=f"den{s}_{c}")
            return av, den

        def avden_tile(avden, s, c, t):
            av, den = avden
            at = ats.pop((s, c, t))
            st, sp = (t == 0), (t == NT - 1)
            a0 = at[:, 0:512].bitcast(BF16) if t in DVESET else at[:, 0:512]
            a1 = at[:, 512:1024].bitcast(BF16) if t in DVESET else at[:, 512:1024]
            nc.tensor.matmul(av[0:64, :], v[:, t, 2 * s, :], a0,
                             start=st, stop=sp)
            nc.tensor.matmul(av[64:128, :], v[:, t, 2 * s + 1, :], a1,
                             start=st, stop=sp)
            nc.tensor.matmul(den[0:64, :], ones, a0, start=st, stop=sp)
            nc.tensor.matmul(den[64:128, :], ones, a1, start=st, stop=sp)

        def normalize(avden, s, c):
            av, den = avden
            rec = recp.tile([128, 512], F32, tag="rec", name=f"rec{s}_{c}")
            nc.vector.reciprocal_approx_fast(out=rec, in_=den)
            nc.vector.tensor_mul(
                out=aoT[:, s, 512 * c:512 * (c + 1)], in0=av, in1=rec)

        def po_tile(it):
            po = psc.tile([128, 1024], F32, tag="sc", name=f"po{it}")
            for half in range(2):
                sl = slice(512 * half, 512 * (half + 1))
                for s in range(NP):
                    nc.tensor.matmul(
                        po[:, sl], aoT[:, s, 128 * it:128 * (it + 1)],
                        wo_sb[:, s, sl], start=(s == 0), stop=(s == NP - 1))
            ot = outp.tile([128, DIM], BF16, tag="out", name=f"ot{it}")
            nc.scalar.copy(out=ot, in_=po)
            nc.sync.dma_start(
                out=out_d.ap()[128 * it:128 * (it + 1), :], in_=ot)

        def unit(cur, prev, fillers):
            # interleave: scores of `cur` with AV/den of `prev`, 3-t groups
            avden = avden_alloc(*prev) if prev else None
            fi = 0
            for g in range(0, NT, 3):
                if fillers and g in (3, 6, 9, 12) and fi < len(fillers):
                    fillers[fi]()
                    fi += 1
                for t in range(g, min(g + 3, NT)):
                    if prev:
                        avden_tile(avden, prev[0], prev[1], t)
                for t in range(g, min(g + 3, NT)):
                    if cur is not None:
                        sc_tile(cur[0], cur[1], t)
            while fi < len(fillers):
                fillers[fi]()
                fi += 1
            if prev:
                normalize(avden, *prev)

        # ---- schedule: pair-major, software-pipelined by one unit ----
        proj_tile(qT, wqs[0], 0, 0)
        proj_tile(qT, wqs[0], 0, 1)
        w_fetch(1)
        proj_tile(kT, wks[0], 0, 0)
        proj_tile(kT, wks[0], 0, 1)

        seq = [(s, c) for s in range(NP) for c in range(NC)]
        prev = None
        vleft = list(range(NT // 2))
        for idx, cur in enumerate(seq):
            s, c = cur
            fillers = []
            if idx == 0:
                fillers = [(lambda j=j: v_tile(j)) for j in vleft[0:5]]
            elif idx == 1:
                fillers = [(lambda j=j: v_tile(j)) for j in vleft[5:8]]
            elif c == 2 and s < NP - 1:
                sn = s + 1
                if sn not in wqs:
                    w_fetch(sn)
                fillers = [lambda sn=sn: proj_tile(qT, wqs[sn], sn, 0),
                           lambda sn=sn: proj_tile(qT, wqs[sn], sn, 1)]
            elif c == 3 and s < NP - 1:
                sn = s + 1
                fillers = [lambda sn=sn: proj_tile(kT, wks[sn], sn, 0),
                           lambda sn=sn: proj_tile(kT, wks[sn], sn, 1)]
            unit(cur, prev, fillers)
            prev = cur
        unit(None, prev, [])          # drain av of the last unit
        for it in range(N // 128):    # epilogue: partial out-projection
            po_tile(it)


def get_program():
    if "nc" not in _CACHE:
        _CACHE["nc"] = _build_program()
    return _CACHE["nc"]


def make_in_maps(x, w_qkv, w_out, b_out):
    bf = ml_dtypes.bfloat16
    w = np.ascontiguousarray(w_qkv, np.float32)
    wo = np.ascontiguousarray(w_out, np.float32)
    xTs = [np.ascontiguousarray(np.asarray(x[bb], np.float32).T).astype(bf)
           for bb in range(B)]
    in_maps = []
    for core in range(NCORES):
        bb, hg = core // 2, core % 2
        cs = slice(512 * hg, 512 * (hg + 1))
        in_maps.append({
            "xT": xTs[bb],
            "wq": np.ascontiguousarray(w[:, 0:1024][:, cs]).astype(bf),
            "wk": np.ascontiguousarray(w[:, 1024:2048][:, cs]).astype(bf),
            "wv": np.ascontiguousarray(w[:, 2048:3072][:, cs]).astype(bf),
            "wo": np.ascontiguousarray(wo[cs, :]).astype(bf),
        })
    return in_maps


def kernel(x, w_qkv, w_out, b_out):
    nc = get_program()
    in_maps = make_in_maps(x, w_qkv, w_out, b_out)
    res = bass_utils.run_bass_kernel_spmd(nc, in_maps, core_ids=list(range(NCORES)))
    bias = np.asarray(b_out, np.float32)[None, :]
    out = np.empty((B, N, DIM), np.float32)
    for bb in range(B):
        out[bb] = (res.results[2 * bb]["out"].astype(np.float32)
                   + res.results[2 * bb + 1]["out"].astype(np.float32) + bias)
    return out
